# revision 47
# baseline (speedup 1.0000x reference)
"""Multi-head attention kernel for 8 Trainium2 NeuronCores.

Problem: B=4, N=2048, C=1024, H=16 heads, d=64, fp32 in/out.
Sharding: core c -> batch c//2, heads (c%2)*8 .. +8  (8 (b,h) pairs per core).
Each core computes full attention for its head slice independently.

Per-core pipeline (all matmuls bf16 with fp32 PSUM accumulation):
  - gpsimd cast-DMA loads Q/K/V as bf16; heads 0-1 individually (so the
    pipeline starts early), heads 2-7 as three big merged DMAs. Q is
    duplicated via DVE broadcast-copies so both PE row-groups can stream it.
  - batched xbar DMA-transposes build Q^T (duplicated on both partition
    halves) and K^T (even key-blocks on partitions 0-63, odd on 64-127).
    All normal-mode DMAs are emitted before all transpose-mode DMAs: the
    scheduler serializes every xbar-mode transition (~10us each).
  - QK^T: row-packed matmul pairs (tile_position (0,0)/(64,0)) compute two
    key-blocks concurrently (contraction d=64 fills half the PE array each).
  - exp on ScalarE over 3-bank granules (F=1536), scale=1/8 fused, bf16 out.
  - PV: V augmented with a ones column (65 cols) so the PV matmul also
    produces the softmax denominators; accumulated over key blocks in PSUM.
  - ctx drain: cast-copy to bf16, batched xbar transpose, reciprocal of the
    sums column, fused normalize-multiply into the staging tile.
  - Staged [128, 512] rows DMA'd to DRAM per query-block at the end.
"""

import numpy as np

import concourse.bass as bass
from concourse import bacc
import concourse.mybir as mybir
import concourse.tile as tile
from concourse.masks import make_identity

F32 = mybir.dt.float32
BF16 = mybir.dt.bfloat16

# Full-problem constants (hardcoded; kernel.py must be self-contained).
B = 4
N = 2048
C = 1024
H_TOTAL = 16
D = 64
N_CORES = 8
H_LOC = 8          # heads per core
C_LOC = H_LOC * D  # 512: dram cols per core
SCALE = 0.125      # 1/sqrt(64)
GRAN = 3           # S granule size in 512-col units (3 PSUM banks)
H_SOLO = 1         # heads loaded individually before the big batch


def build_nc(h_loc=H_LOC, n_q=N, n_k=N):
    """Build the single-core Bass program (SPMD: same NEFF on all 8 cores)."""
    nc = bacc.Bacc("TRN2", target_bir_lowering=False)

    qb_n = n_q // 128          # query blocks
    kb_n = n_k // 128          # key blocks
    kbp_n = kb_n // 2          # key block pairs
    qq_n = n_q // 512          # query chunks of 512
    c_loc = h_loc * D
    h_solo = min(H_SOLO, h_loc)
    h_rest = h_loc - h_solo

    q_d = nc.dram_tensor("query_layer", [n_q, c_loc], BF16, kind="ExternalInput")
    k_d = nc.dram_tensor("key_layer", [n_k, c_loc], BF16, kind="ExternalInput")
    v_d = nc.dram_tensor("value_layer", [n_k, c_loc], BF16, kind="ExternalInput")
    o_d = nc.dram_tensor("out", [n_q, c_loc], F32, kind="ExternalOutput")

    def dram_src(t, h0, nh):
        # [p, h, blk, d] view of heads h0..h0+nh of a [n, c_loc] dram tensor
        return t[:, h0 * D:(h0 + nh) * D].rearrange(
            "(blk p) (h d) -> p h blk d", p=128, h=nh)

    with tile.TileContext(nc) as tc:
        with (
            tc.tile_pool(name="persist", bufs=1) as persist,
            tc.tile_pool(name="ppool", bufs=4) as ppool,

            tc.tile_pool(name="trsbp", bufs=6) as trsbp,
            tc.tile_pool(name="rpool", bufs=6) as rpool,
            tc.tile_pool(name="spool", bufs=2, space="PSUM") as spool,
            tc.tile_pool(name="ctxps", bufs=2, space="PSUM") as ctxps,
        ):
            # persistent per-core input tiles (merged across heads)
            qn = persist.tile([128, h_loc, qb_n, 2, D], BF16, name="qn")
            kn = persist.tile([128, h_loc, kb_n, D], BF16, name="kn")
            va = persist.tile([128, h_loc, kb_n, D + 1], BF16, name="va")
            q2t = persist.tile([128, h_loc, qb_n, 128], BF16, name="q2t")
            k2t = persist.tile([128, h_loc, kbp_n, 128], BF16, name="k2t")
            q1 = persist.tile([128, h_loc, qb_n, D], BF16, name="q1")

            # prep is emitted in two slices to bound xbar-mode flips (3
            # total) while letting head-0/1 compute start early:
            # [casts h<2] [xposes h<2] [casts h>=2] [xposes h>=2]
            def cast_head(h):
                nc.sync.dma_start(out=kn[:, h], in_=dram_src(k_d, h, 1)[:, 0])
                nc.sync.dma_start(out=q1[:, h], in_=dram_src(q_d, h, 1)[:, 0])

                def dup(b0, b1):
                    q1h = q1[:, h, b0:b1]
                    q1_dup = bass.AP(
                        tensor=q1h.tensor,
                        offset=q1h.offset,
                        ap=[q1h.ap[0], q1h.ap[1], [0, 2], q1h.ap[2]],
                    )
                    nc.vector.tensor_copy(qn[:, h, b0:b1], q1_dup)

                if h == 0 and qb_n > 4:
                    # head 0: duplicate the first 4 query blocks separately
                    # so the PE bootstrap's first Q group starts earlier
                    dup(0, 4)
                    dup(4, qb_n)
                else:
                    dup(0, qb_n)

            def load_v(h):
                nc.sync.dma_start(out=va[:, h, :, 0:D],
                                  in_=dram_src(v_d, h, 1)[:, 0])
                nc.vector.memset(va[:, h, :, D], 1.0)

            def xpose_head(h):
                nc.sync.dma_start_transpose(q2t[:, h], qn[:, h])
                nc.sync.dma_start_transpose(k2t[:, h], kn[:, h])

            # ACT table preload: a dummy exp so the ~1.3us table load
            # happens during the prefix, off the critical path
            tiny = persist.tile([1, 8], F32, name="tiny")
            nc.vector.memset(tiny, 0.0)
            tiny2 = persist.tile([1, 8], F32, name="tiny2")
            nc.scalar.activation(tiny2, tiny,
                                 mybir.ActivationFunctionType.Exp)

            ident = persist.tile([128, 128], BF16, name="ident")
            make_identity(nc, ident)

            # ring of drain staging tiles; rows 64:80 are xbar padding and
            # only need zeroing once (the per-drain copy never touches them)
            ctxt_ring = [persist.tile([80, 512], BF16, name=f"ctxt{i}")
                         for i in range(6)]
            for i, t in enumerate(ctxt_ring):
                # tiles 0-1 are fully zeroed (the HAM warm-up matmuls read
                # them); the rest only need the xbar padding rows
                if i < 2:
                    nc.vector.memset(t, 0.0)
                else:
                    nc.vector.memset(t[64:80, :], 0.0)

            def pe_xpose_head(h):
                """Bootstrap transposes on the (idle) PE for head 0 --
                avoids any DMA xbar-mode flip on the critical path. The
                first K/Q block groups go first so QK can start ASAP."""
                k1 = min(2, kbp_n)
                q1g = min(4, qb_n)
                order = [("k", 0, k1), ("q", 0, q1g)]
                if kbp_n > k1:
                    order.append(("k", k1, kbp_n - k1))
                if qb_n > q1g:
                    for g0 in range(q1g, qb_n, 8):
                        order.append(("q", g0, min(8, qb_n - g0)))
                for kind, g0, grp in order:
                    tp = ctxps.tile([128, grp, 128], BF16,
                                    name="tboot", tag="ctx")
                    for j in range(grp):
                        if kind == "q":
                            blk_in = qn[:, h, g0 + j, :, :]
                            dst = q2t
                        else:
                            blk_in = kn[:, h, (g0 + j) * 2:(g0 + j) * 2 + 2, :]
                            dst = k2t
                        nc.tensor.transpose(tp[:, j, :], blk_in, ident)
                    if kind == "q":
                        nc.vector.tensor_copy(q2t[:, h, g0:g0 + grp, :], tp)
                    else:
                        nc.vector.tensor_copy(k2t[:, h, g0:g0 + grp, :], tp)

            # HAM warm-up: ~4us of dummy matmuls on zeroed tiles while the
            # first loads are in flight, so real QKs start at 2.4GHz
            warm = spool.tile([128, GRAN * 512], F32, name="sgran")
            for w in range(10):
                nc.tensor.matmul(
                    warm[:, 0:512],
                    lhsT=ctxt_ring[1][0:64, 0:128],
                    rhs=ctxt_ring[0][0:64, :],
                    start=True, stop=True)

            # phase A: per-head loads; head 0 transposed on the (idle) PE.
            # Heads 1-3 load before the single flip so their xposes run in
            # one early transpose batch; heads 4+ go through one giant
            # xpose pair afterwards (mode flips stay at 3).
            n_early = min(4, h_loc)
            for h in range(min(2, h_loc)):
                cast_head(h)
                load_v(h)
            for h in range(2, n_early):
                cast_head(h)
                load_v(h)
            pe_xpose_head(0)
            for h in range(1, n_early):
                xpose_head(h)
            for h in range(n_early, h_loc):
                cast_head(h)
                load_v(h)
            if h_loc > n_early:
                # one giant xbar transpose per tensor covers heads 4..h_loc
                nc.sync.dma_start_transpose(
                    q2t[:, n_early:].rearrange("p h b f -> p (h b) f"),
                    qn[:, n_early:])
                nc.sync.dma_start_transpose(
                    k2t[:, n_early:].rearrange("p h b f -> p (h b) f"),
                    kn[:, n_early:])

            # output staging: [128, qb, c] so one fused normalize-mul can
            # write 4 query blocks at once
            outst = persist.tile([128, qb_n, c_loc], F32, name="outst")

            # ---- main loop: global stream of 512-col (h, qq, kb) units ----
            units = [(h, qq, kb)
                     for h in range(h_loc)
                     for qq in range(qq_n)
                     for kb in range(kb_n)]

            drain_count = [0]
            pending = []

            def drain(h, qq):
                """Copy ctx^T, reciprocal the sums row in place (partition
                64 -> 64), transpose, then normalize-multiply on gpsimd.
                The DVE only does the PSUM-freeing copy+reciprocal, so the
                ctx slot release never waits on the SP transpose
                round-trip; gpsimd (idle) absorbs the transpose-dependent
                work."""
                ctx = ctx_tiles.pop((h, qq))
                ctxt = ctxt_ring[drain_count[0] % len(ctxt_ring)]
                drain_count[0] += 1
                nc.vector.tensor_copy(ctxt[0:64, :], ctx[0:64, :])
                with nc.allow_low_precision("softmax denom fits bf16"):
                    nc.vector.reciprocal(ctxt[64:65, :], ctx[64:65, :])
                trsb = trsbp.tile([128, 4, 80], BF16, name="trsb")
                nc.sync.dma_start_transpose(trsb, ctxt)
                rs_b = bass.AP(
                    tensor=trsb.tensor,
                    offset=trsb.offset + D,
                    ap=[trsb.ap[0], [80, 4], [0, D]],
                )
                last = (h == h_loc - 1 and qq == qq_n - 1)
                eng = nc.vector if last else nc.gpsimd
                eng.tensor_tensor(
                    out=outst[:, qq * 4:qq * 4 + 4, h * D:(h + 1) * D],
                    in0=trsb[:, :, 0:D],
                    in1=rs_b,
                    op=mybir.AluOpType.mult,
                )

            ctx_tiles = {}
            n_units = len(units)
            u = 0
            while u < n_units:
                group = units[u:u + GRAN]
                g = len(group)
                gr = spool.tile([128, GRAN * 512], F32, name="sgran")
                psb = ppool.tile([128, GRAN * 512], BF16, name="p")
                # QK matmuls for the group (kb pairs stay emission-adjacent)
                for j, (h, qq, kb) in enumerate(group):
                    half = kb % 2
                    nc.tensor.matmul(
                        gr[:, j * 512:(j + 1) * 512],
                        lhsT=k2t[half * 64:half * 64 + 64, h, kb // 2, :],
                        rhs=q2t[half * 64:half * 64 + 64, h,
                                qq * 4:qq * 4 + 4, :],
                        start=True, stop=True,
                        tile_position=(half * 64, 0))
                # exp over the whole granule
                nc.scalar.activation(psb[:, 0:g * 512], gr[:, 0:g * 512],
                                     mybir.ActivationFunctionType.Exp,
                                     scale=SCALE)
                # PV accumulation per unit
                for j, (h, qq, kb) in enumerate(group):
                    if kb == 0:
                        ctx_tiles[(h, qq)] = ctxps.tile(
                            [D + 1, 512], F32, name="ctx")
                    nc.tensor.matmul(
                        ctx_tiles[(h, qq)],
                        lhsT=va[:, h, kb, :],
                        rhs=psb[:, j * 512:(j + 1) * 512],
                        start=(kb == 0), stop=(kb == kb_n - 1))
                    if kb == kb_n - 1:
                        drain(h, qq)
                        if h == h_loc - 1:
                            nc.sync.dma_start(
                                out=o_d[qq * 512:(qq + 1) * 512, :].rearrange(
                                    "(b p) c -> p b c", p=128),
                                in_=outst[:, qq * 4:qq * 4 + 4, :])
                u += g

    nc.finalize()
    return nc


_NC_CACHE = {}


def _get_nc():
    if "nc" not in _NC_CACHE:
        _NC_CACHE["nc"] = build_nc()
    return _NC_CACHE["nc"]


def _shard(x, c, dtype):
    b = c // 2
    cs = (c % 2) * C_LOC
    return np.ascontiguousarray(x[b, :, cs:cs + C_LOC]).astype(dtype)


def run_spmd(query_layer, key_layer, value_layer, **kwargs):
    """Run on 8 cores; returns (full_output, BassKernelResults)."""
    from concourse.bass_utils import run_bass_kernel_spmd

    q = np.asarray(query_layer, dtype=np.float32)
    k = np.asarray(key_layer, dtype=np.float32)
    v = np.asarray(value_layer, dtype=np.float32)
    import ml_dtypes
    bf16 = ml_dtypes.bfloat16
    in_maps = [
        {"query_layer": _shard(q, c, bf16), "key_layer": _shard(k, c, bf16),
         "value_layer": _shard(v, c, bf16)}
        for c in range(N_CORES)
    ]
    nc = _get_nc()
    res = run_bass_kernel_spmd(nc, in_maps, core_ids=list(range(N_CORES)),
                               **kwargs)
    out = np.empty((B, N, C), dtype=np.float32)
    for c in range(N_CORES):
        b = c // 2
        cs = (c % 2) * C_LOC
        out[b, :, cs:cs + C_LOC] = res.results[c]["out"]
    return out, res


def kernel(query_layer, key_layer, value_layer):
    out, _ = run_spmd(query_layer, key_layer, value_layer)
    return out


# revision 49
# speedup vs baseline: 1.0079x; 1.0079x over previous
"""Multi-head attention kernel for 8 Trainium2 NeuronCores.

Problem: B=4, N=2048, C=1024, H=16 heads, d=64, fp32 in/out.
Sharding: core c -> batch c//2, heads (c%2)*8 .. +8  (8 (b,h) pairs per core).
Each core computes full attention for its head slice independently.

Per-core pipeline (all matmuls bf16 with fp32 PSUM accumulation):
  - gpsimd cast-DMA loads Q/K/V as bf16; heads 0-1 individually (so the
    pipeline starts early), heads 2-7 as three big merged DMAs. Q is
    duplicated via DVE broadcast-copies so both PE row-groups can stream it.
  - batched xbar DMA-transposes build Q^T (duplicated on both partition
    halves) and K^T (even key-blocks on partitions 0-63, odd on 64-127).
    All normal-mode DMAs are emitted before all transpose-mode DMAs: the
    scheduler serializes every xbar-mode transition (~10us each).
  - QK^T: row-packed matmul pairs (tile_position (0,0)/(64,0)) compute two
    key-blocks concurrently (contraction d=64 fills half the PE array each).
  - exp on ScalarE over 3-bank granules (F=1536), scale=1/8 fused, bf16 out.
  - PV: V augmented with a ones column (65 cols) so the PV matmul also
    produces the softmax denominators; accumulated over key blocks in PSUM.
  - ctx drain: cast-copy to bf16, batched xbar transpose, reciprocal of the
    sums column, fused normalize-multiply into the staging tile.
  - Staged [128, 512] rows DMA'd to DRAM per query-block at the end.
"""

import numpy as np

import concourse.bass as bass
from concourse import bacc
import concourse.mybir as mybir
import concourse.tile as tile
from concourse.masks import make_identity

F32 = mybir.dt.float32
BF16 = mybir.dt.bfloat16

# Full-problem constants (hardcoded; kernel.py must be self-contained).
B = 4
N = 2048
C = 1024
H_TOTAL = 16
D = 64
N_CORES = 8
H_LOC = 8          # heads per core
C_LOC = H_LOC * D  # 512: dram cols per core
SCALE = 0.125      # 1/sqrt(64)
GRAN = 3           # S granule size in 512-col units (3 PSUM banks)
H_SOLO = 1         # heads loaded individually before the big batch


def build_nc(h_loc=H_LOC, n_q=N, n_k=N):
    """Build the single-core Bass program (SPMD: same NEFF on all 8 cores)."""
    nc = bacc.Bacc("TRN2", target_bir_lowering=False)

    qb_n = n_q // 128          # query blocks
    kb_n = n_k // 128          # key blocks
    kbp_n = kb_n // 2          # key block pairs
    qq_n = n_q // 512          # query chunks of 512
    c_loc = h_loc * D
    h_solo = min(H_SOLO, h_loc)
    h_rest = h_loc - h_solo

    q_d = nc.dram_tensor("query_layer", [n_q, c_loc], BF16, kind="ExternalInput")
    k_d = nc.dram_tensor("key_layer", [n_k, c_loc], BF16, kind="ExternalInput")
    v_d = nc.dram_tensor("value_layer", [n_k, c_loc], BF16, kind="ExternalInput")
    o_d = nc.dram_tensor("out", [n_q, c_loc], F32, kind="ExternalOutput")

    def dram_src(t, h0, nh):
        # [p, h, blk, d] view of heads h0..h0+nh of a [n, c_loc] dram tensor
        return t[:, h0 * D:(h0 + nh) * D].rearrange(
            "(blk p) (h d) -> p h blk d", p=128, h=nh)

    with tile.TileContext(nc) as tc:
        with (
            tc.tile_pool(name="persist", bufs=1) as persist,
            tc.tile_pool(name="ppool", bufs=4) as ppool,

            tc.tile_pool(name="trsbp", bufs=6) as trsbp,
            tc.tile_pool(name="rpool", bufs=6) as rpool,
            tc.tile_pool(name="spool", bufs=2, space="PSUM") as spool,
            tc.tile_pool(name="ctxps", bufs=2, space="PSUM") as ctxps,
        ):
            # persistent per-core input tiles (merged across heads)
            qn = persist.tile([128, h_loc, qb_n, 2, D], BF16, name="qn")
            kn = persist.tile([128, h_loc, kb_n, D], BF16, name="kn")
            va = persist.tile([128, h_loc, kb_n, D + 1], BF16, name="va")
            q2t = persist.tile([128, h_loc, qb_n, 128], BF16, name="q2t")
            k2t = persist.tile([128, h_loc, kbp_n, 128], BF16, name="k2t")
            q1 = persist.tile([128, h_loc, qb_n, D], BF16, name="q1")

            # prep is emitted in two slices to bound xbar-mode flips (3
            # total) while letting head-0/1 compute start early:
            # [casts h<2] [xposes h<2] [casts h>=2] [xposes h>=2]
            def cast_head(h):
                nc.sync.dma_start(out=kn[:, h], in_=dram_src(k_d, h, 1)[:, 0])
                nc.sync.dma_start(out=q1[:, h], in_=dram_src(q_d, h, 1)[:, 0])

                def dup(b0, b1):
                    q1h = q1[:, h, b0:b1]
                    q1_dup = bass.AP(
                        tensor=q1h.tensor,
                        offset=q1h.offset,
                        ap=[q1h.ap[0], q1h.ap[1], [0, 2], q1h.ap[2]],
                    )
                    nc.vector.tensor_copy(qn[:, h, b0:b1], q1_dup)

                if h == 0 and qb_n > 4:
                    # head 0: duplicate the first 4 query blocks separately
                    # so the PE bootstrap's first Q group starts earlier
                    dup(0, 4)
                    dup(4, qb_n)
                else:
                    dup(0, qb_n)

            def load_v(h):
                nc.sync.dma_start(out=va[:, h, :, 0:D],
                                  in_=dram_src(v_d, h, 1)[:, 0])
                nc.vector.memset(va[:, h, :, D], 1.0)

            def xpose_head(h):
                nc.sync.dma_start_transpose(q2t[:, h], qn[:, h])
                nc.sync.dma_start_transpose(k2t[:, h], kn[:, h])

            # ACT table preload: a dummy exp so the ~1.3us table load
            # happens during the prefix, off the critical path
            tiny = persist.tile([1, 8], F32, name="tiny")
            nc.vector.memset(tiny, 0.0)
            tiny2 = persist.tile([1, 8], F32, name="tiny2")
            nc.scalar.activation(tiny2, tiny,
                                 mybir.ActivationFunctionType.Exp)

            ident = persist.tile([128, 128], BF16, name="ident")
            make_identity(nc, ident)

            # ring of drain staging tiles; rows 64:80 are xbar padding and
            # only need zeroing once (the per-drain copy never touches them)
            ctxt_ring = [persist.tile([80, 512], BF16, name=f"ctxt{i}")
                         for i in range(6)]
            for i, t in enumerate(ctxt_ring):
                # tiles 0-1 are fully zeroed (the HAM warm-up matmuls read
                # them); the rest only need the xbar padding rows
                if i < 2:
                    nc.vector.memset(t, 0.0)
                else:
                    nc.vector.memset(t[64:80, :], 0.0)

            def pe_xpose_head(h):
                """Bootstrap transposes on the (idle) PE for head 0 --
                avoids any DMA xbar-mode flip on the critical path. The
                first K/Q block groups go first so QK can start ASAP."""
                k1 = min(2, kbp_n)
                q1g = min(4, qb_n)
                order = [("k", 0, k1), ("q", 0, q1g)]
                if kbp_n > k1:
                    order.append(("k", k1, kbp_n - k1))
                if qb_n > q1g:
                    for g0 in range(q1g, qb_n, 8):
                        order.append(("q", g0, min(8, qb_n - g0)))
                for kind, g0, grp in order:
                    tp = ctxps.tile([128, grp, 128], BF16,
                                    name="tboot", tag="ctx")
                    for j in range(grp):
                        if kind == "q":
                            blk_in = qn[:, h, g0 + j, :, :]
                            dst = q2t
                        else:
                            blk_in = kn[:, h, (g0 + j) * 2:(g0 + j) * 2 + 2, :]
                            dst = k2t
                        nc.tensor.transpose(tp[:, j, :], blk_in, ident)
                    if kind == "q":
                        nc.vector.tensor_copy(q2t[:, h, g0:g0 + grp, :], tp)
                    else:
                        nc.vector.tensor_copy(k2t[:, h, g0:g0 + grp, :], tp)

            # HAM warm-up: ~4us of dummy matmuls on zeroed tiles while the
            # first loads are in flight, so real QKs start at 2.4GHz
            warm = spool.tile([128, GRAN * 512], F32, name="sgran")
            for w in range(10):
                nc.tensor.matmul(
                    warm[:, 0:512],
                    lhsT=ctxt_ring[1][0:64, 0:128],
                    rhs=ctxt_ring[0][0:64, :],
                    start=True, stop=True)

            # phase A: per-head loads; head 0 transposed on the (idle) PE.
            # Heads 1-3 load before the single flip so their xposes run in
            # one early transpose batch; heads 4+ go through one giant
            # xpose pair afterwards (mode flips stay at 3).
            n_early = min(4, h_loc)
            for h in range(min(2, h_loc)):
                cast_head(h)
                load_v(h)
            for h in range(2, n_early):
                cast_head(h)
                load_v(h)
            pe_xpose_head(0)
            for h in range(1, n_early):
                xpose_head(h)
            for h in range(n_early, h_loc):
                cast_head(h)
                load_v(h)
            if h_loc > n_early:
                # one giant xbar transpose per tensor covers heads 4..h_loc
                nc.sync.dma_start_transpose(
                    q2t[:, n_early:].rearrange("p h b f -> p (h b) f"),
                    qn[:, n_early:])
                nc.sync.dma_start_transpose(
                    k2t[:, n_early:].rearrange("p h b f -> p (h b) f"),
                    kn[:, n_early:])

            # output staging: [128, qb, c] so one fused normalize-mul can
            # write 4 query blocks at once
            outst = persist.tile([128, qb_n, c_loc], F32, name="outst")

            # ---- main loop: global stream of 512-col (h, qq, kb) units ----
            units = [(h, qq, kb)
                     for h in range(h_loc)
                     for qq in range(qq_n)
                     for kb in range(kb_n)]

            drain_count = [0]
            pending = []

            def drain(h, qq):
                """Copy ctx^T, reciprocal the sums row in place (partition
                64 -> 64), transpose, then normalize-multiply on gpsimd.
                The DVE only does the PSUM-freeing copy+reciprocal, so the
                ctx slot release never waits on the SP transpose
                round-trip; gpsimd (idle) absorbs the transpose-dependent
                work."""
                ctx = ctx_tiles.pop((h, qq))
                ctxt = ctxt_ring[drain_count[0] % len(ctxt_ring)]
                drain_count[0] += 1
                nc.vector.tensor_copy(ctxt[0:64, :], ctx[0:64, :])
                with nc.allow_low_precision("softmax denom fits bf16"):
                    nc.vector.reciprocal(ctxt[64:65, :], ctx[64:65, :])
                trsb = trsbp.tile([128, 4, 80], BF16, name="trsb")
                nc.sync.dma_start_transpose(trsb, ctxt)
                rs_b = bass.AP(
                    tensor=trsb.tensor,
                    offset=trsb.offset + D,
                    ap=[trsb.ap[0], [80, 4], [0, D]],
                )
                last = (h == h_loc - 1 and qq == qq_n - 1)
                eng = nc.vector if last else nc.gpsimd
                eng.tensor_tensor(
                    out=outst[:, qq * 4:qq * 4 + 4, h * D:(h + 1) * D],
                    in0=trsb[:, :, 0:D],
                    in1=rs_b,
                    op=mybir.AluOpType.mult,
                )

            ctx_tiles = {}
            n_units = len(units)
            u = 0
            while u < n_units:
                group = units[u:u + GRAN]
                g = len(group)
                gr = spool.tile([128, GRAN * 512], F32, name="sgran")
                psb = ppool.tile([128, GRAN * 512], BF16, name="p")
                # QK matmuls for the group (kb pairs stay emission-adjacent)
                for j, (h, qq, kb) in enumerate(group):
                    half = kb % 2
                    nc.tensor.matmul(
                        gr[:, j * 512:(j + 1) * 512],
                        lhsT=k2t[half * 64:half * 64 + 64, h, kb // 2, :],
                        rhs=q2t[half * 64:half * 64 + 64, h,
                                qq * 4:qq * 4 + 4, :],
                        start=True, stop=True,
                        tile_position=(half * 64, 0))
                # exp over the whole granule
                nc.scalar.activation(psb[:, 0:g * 512], gr[:, 0:g * 512],
                                     mybir.ActivationFunctionType.Exp,
                                     scale=SCALE)
                # PV accumulation per unit
                for j, (h, qq, kb) in enumerate(group):
                    if kb == 0:
                        ctx_tiles[(h, qq)] = ctxps.tile(
                            [D + 1, 512], F32, name="ctx")
                    nc.tensor.matmul(
                        ctx_tiles[(h, qq)],
                        lhsT=va[:, h, kb, :],
                        rhs=psb[:, j * 512:(j + 1) * 512],
                        start=(kb == 0), stop=(kb == kb_n - 1))
                    if kb == kb_n - 1:
                        drain(h, qq)
                        if h == h_loc - 1:
                            nc.sync.dma_start(
                                out=o_d[qq * 512:(qq + 1) * 512, :].rearrange(
                                    "(b p) c -> p b c", p=128),
                                in_=outst[:, qq * 4:qq * 4 + 4, :])
                u += g

    nc.finalize()
    return nc


_NC_CACHE = {}


def _get_nc():
    if "nc" not in _NC_CACHE:
        _NC_CACHE["nc"] = build_nc()
    return _NC_CACHE["nc"]


def _shard(x, c, dtype):
    b = c // 2
    cs = (c % 2) * C_LOC
    return np.ascontiguousarray(x[b, :, cs:cs + C_LOC]).astype(dtype)


def run_spmd(query_layer, key_layer, value_layer, **kwargs):
    """Run on 8 cores; returns (full_output, BassKernelResults)."""
    from concourse.bass_utils import run_bass_kernel_spmd

    q = np.asarray(query_layer, dtype=np.float32)
    k = np.asarray(key_layer, dtype=np.float32)
    v = np.asarray(value_layer, dtype=np.float32)
    import ml_dtypes
    bf16 = ml_dtypes.bfloat16
    in_maps = [
        {"query_layer": _shard(q, c, bf16), "key_layer": _shard(k, c, bf16),
         "value_layer": _shard(v, c, bf16)}
        for c in range(N_CORES)
    ]
    nc = _get_nc()
    res = run_bass_kernel_spmd(nc, in_maps, core_ids=list(range(N_CORES)),
                               **kwargs)
    out = np.empty((B, N, C), dtype=np.float32)
    for c in range(N_CORES):
        b = c // 2
        cs = (c % 2) * C_LOC
        out[b, :, cs:cs + C_LOC] = res.results[c]["out"]
    return out, res


def kernel(query_layer, key_layer, value_layer):
    out, _ = run_spmd(query_layer, key_layer, value_layer)
    return out
